# revision 1
# baseline (speedup 1.0000x reference)
"""Trainium2 Bass kernel for nn_BasicNCAModel (neural cellular automaton).

Sharding: data-parallel over batch B=8 across 8 NeuronCores (1 image/core).
kernel() takes full inputs, shards per image on the host, runs the SPMD Bass
module via run_bass_kernel_spmd (PJRT under axon), and reassembles.

Per-core design (hardcoded for B=8, H=W=128, C=24, hidden=128, steps=8):
  - x (f32 master) is channel-major with a halo: partition 32g+c holds
    channel c of image rows [32g-1, 32g+32] (4 row-groups, 34 rows x 132
    pitch), so circular padding becomes plain address offsets. A fp16
    shadow (x16, ping-ponged per step) feeds the tensor engine.
  - perceive + W1 fuse into per-tap matrices A_t[k,c] = W1[k,24+c]*w1[t,c]
    + W1[k,48+c]*w2[t,c] (+W1[k,c] at the center tap). Per group a fp16
    "dx-stack" holds rows 24d+c = x16 shifted by dx=d-1 (shift baked into
    a contiguous DMA copy), row 72 = fire = (u<0.5), rows 73..127 = zero
    so K=128 (full-K matmuls run ~2x faster than partial-K). mm1 is then
    3 matmuls per 512-pixel tile (dy in {-1,0,1} via +-PITCH in the rhs
    AP); the center one adds fire*M (M=512). Evacuation (ScalarE relu +
    VectorE STT, split) computes h' = relu(h + b1 - M): masked pixels get
    relu(h-M)=0 so dx=0 exactly (fire trick).
  - mm2 (fp16) is column-tiled: group g's dx lands at PSUM partitions
    32g..32g+31 (W2^T with channels 0..3 zeroed for the channel mask),
    so x += dx is one full-width f32 DVE add per tile. DVE also refreshes
    the x16 shadow; stack rebuild DMAs are sliced and issued as soon as
    the casts they need complete (j-order [2,3,1,4,0,5,6,7] keeps the
    cross-step dependencies off the tensor-engine critical path); stack
    halo rows come from the neighbor group's stack (same partitions, DVE).
"""

import sys

if "/opt/trn_rl_repo" not in sys.path:
    sys.path.insert(0, "/opt/trn_rl_repo")

import numpy as np

C = 24
NIC = 4
H = 128
WID = 128
HID = 128
STEPS = 8
B = 8
G = 4          # row groups
RG = 32        # image rows per group
PITCH = 132    # free-dim row pitch (130 used + 2 pad)
GROWS = 34     # rows incl halo
FB = GROWS * PITCH
TW = 512       # pixel tile = 4 image rows * 128 cols
JT = RG // 4   # tiles per group per step
M_FIRE = 512.0

_CACHE = {}


def _build_module():
    from concourse import bacc, mybir, tile

    f32 = mybir.dt.float32
    f16 = mybir.dt.float16
    Alu = mybir.AluOpType
    Act = mybir.ActivationFunctionType

    nc = bacc.Bacc(
        "TRN2",
        target_bir_lowering=False,
        debug=False,
        enable_asserts=False,
        num_devices=8,
    )

    xin = nc.dram_tensor("xin", [128, FB], f32, kind="ExternalInput").ap()
    uin = nc.dram_tensor("uin", [128, 4096], f32, kind="ExternalInput").ap()
    apack = nc.dram_tensor("apack", [128, 384], f16, kind="ExternalInput").ap()
    w2p = nc.dram_tensor("w2p", [128, 32], f16, kind="ExternalInput").ap()
    b1col = nc.dram_tensor("b1col", [128, 1], f32, kind="ExternalInput").ap()
    # host-prebuilt initial fp16 shadow and step-0 stacks
    x16in = nc.dram_tensor("x16in", [128, FB], f16, kind="ExternalInput").ap()
    stkin = nc.dram_tensor("stkin", [128, 4 * FB], f16, kind="ExternalInput").ap()
    xout = nc.dram_tensor("xout", [128, 4096], f32, kind="ExternalOutput").ap()

    with tile.TileContext(nc) as tc:
        import contextlib

        with contextlib.ExitStack() as ctx:
            sing = ctx.enter_context(tc.tile_pool(name="sing", bufs=1))
            hpool = ctx.enter_context(tc.tile_pool(name="h", bufs=6, space="PSUM"))
            dxpool = ctx.enter_context(tc.tile_pool(name="dx", bufs=2, space="PSUM"))
            hsb = ctx.enter_context(tc.tile_pool(name="hsb", bufs=8))

            xa = sing.tile([128, FB], f32)
            xb = sing.tile([128, FB], f32)
            x16a = sing.tile([128, FB], f16)
            x16b = sing.tile([128, FB], f16)
            usb = sing.tile([128, 4096], f32)
            fire = sing.tile([128, 4096], f16)
            A = sing.tile([128, 384], f16)
            W2s = sing.tile([128, 32], f16)
            zeros = sing.tile([128, TW // 2], f32)
            b1c = sing.tile([128, 1], f32)
            # dx-shift stacks: rows 24d+c = channel c shifted by dx=d-1;
            # row 72 = fire; rows 73..127 zero (pads K to 128 for full-rate
            # matmuls). One per group, ping-ponged per step.
            stk = [
                [sing.tile([128, FB], f16, name=f"stk_{g}_{b}") for b in range(2)]
                for g in range(G)
            ]

            # stacks + weights first (taps need only these), spread queues
            _ld = [nc.sync, nc.scalar, nc.gpsimd, nc.sync]
            for g in range(G):
                _ld[g].dma_start(
                    stk[g][0][:, 0:2376],
                    stkin[:, g * FB : g * FB + 2376],
                )
            for g in range(G):
                _ld[(g + 1) % 3].dma_start(
                    stk[g][0][:, 2376:FB],
                    stkin[:, g * FB + 2376 : (g + 1) * FB],
                )
            nc.scalar.dma_start(A[:], apack[:])
            nc.gpsimd.dma_start(W2s[:], w2p[:])
            nc.gpsimd.dma_start(b1c[:], b1col[:])
            nc.sync.dma_start(x16a[:], x16in[:])
            nc.sync.dma_start(xa[:], xin[:])
            nc.scalar.dma_start(usb[:], uin[:])
            for g in range(G):
                # odd-step stacks: zero rows 72.. (fire pads + K-padding)
                nc.vector.memset(stk[g][1][64:128, :], 0.0)
            nc.vector.memset(zeros[:], 0.0)
            nc.vector.memset(x16b[:], 0.0)
            # fire = (u < 0.5) as 0.0/1.0 (exact f32 compare, f16 out)
            nc.vector.tensor_scalar(fire[:], usb[:], 0.5, None, Alu.is_lt)

            xf32 = [xa, xb]
            xf16 = [x16a, x16b]
            fire3 = fire[:].rearrange("p (r w) -> p r w", w=128)
            # stack slice boundaries (flat y): rows 1-4 | 5-16 | 17-24 | 25-32
            SLICES = [(132, 660), (660, 2244), (2244, 3300), (3300, 4356)]
            ISSUE = None  # set per step

            def emit_slice(s, sl, colhalo_rows):
                """Col-halos for the rows then the slice copy for step s+1."""
                x6v = xf16[(s + 1) % 2][:].rearrange("p (r w) -> p r w", w=PITCH)
                lo, hi = colhalo_rows
                for g in range(G):
                    p0 = 32 * g
                    nc.vector.tensor_copy(
                        x6v[p0 : p0 + 24, lo:hi, 0:1],
                        x6v[p0 : p0 + 24, lo:hi, 128:129],
                    )
                    nc.vector.tensor_copy(
                        x6v[p0 : p0 + 24, lo:hi, 129:130],
                        x6v[p0 : p0 + 24, lo:hi, 1:2],
                    )
                x6 = xf16[(s + 1) % 2]
                ylo, yhi = SLICES[sl]
                for g in range(G):
                    sg = stk[g][(s + 1) % 2]
                    for d in range(3):
                        eng = ISSUE[(g + d) % len(ISSUE)]
                        eng.dma_start(
                            sg[24 * d : 24 * d + 24, ylo:yhi],
                            x6[32 * g : 32 * g + 24, ylo + d : yhi + d],
                        )

            def emit_fire(s):
                """Fire rows for step s+1 (no cast deps; only WAR on s-1)."""
                sb = (s + 1) % 2
                for g in range(G):
                    s3 = stk[g][sb][:].rearrange("p (r w) -> p r w", w=PITCH)
                    nc.gpsimd.dma_start(
                        s3[72:73, 1:33, 0:128],
                        fire3[32 * g + s + 1 : 32 * g + s + 2, :, :],
                    )

            def emit_fire_edges(s):
                """Neighbor-stack halo rows for step s+1."""
                sb = (s + 1) % 2
                for g in range(G):
                    sg = stk[g][sb]
                    sm = stk[(g - 1) % G][sb]
                    sp = stk[(g + 1) % G][sb]
                    nc.vector.tensor_copy(sg[:73, 0:132], sm[:73, 4224:4356])
                    nc.vector.tensor_copy(sg[:73, 4356:4488], sp[:73, 132:264])

            JORD = [2, 3, 1, 4, 0, 5, 6, 7]
            # stack slice sl becomes buildable once these casts are done
            TRIGGER = {0: 0, 1: 1, 5: 2, 7: 3}  # cast j -> slice index
            for s in range(STEPS):
                ISSUE = [nc.sync, nc.gpsimd]
                if s + 1 < STEPS:
                    emit_fire(s)
                xc = xf32[s % 2][:].rearrange("p (r w) -> p r w", w=PITCH)
                xn = xf32[(s + 1) % 2][:].rearrange("p (r w) -> p r w", w=PITCH)
                xn6 = xf16[(s + 1) % 2][:].rearrange("p (r w) -> p r w", w=PITCH)
                stks = [
                    stk[g][s % 2][:].rearrange("p (r w) -> p r w", w=PITCH)
                    for g in range(G)
                ]

                def mm2_update(j, hss):
                    r0 = 4 * j + 1
                    dxt = dxpool.tile(
                        [128, TW], f32, tag="dx", name=f"dx_{s}_{j}"
                    )
                    for g in range(G):
                        nc.tensor.matmul(
                            dxt[32 * g : 32 * g + 32, :],
                            W2s[:],
                            hss[g][:],
                            start=True,
                            stop=True,
                            tile_position=(0, 32 * g),
                        )
                    # x_next = x + dx  (dx rows 24..31 of each band are 0)
                    dx3 = dxt[:].rearrange("p (a b) -> p a b", b=128)
                    nc.vector.tensor_tensor(
                        xn[:, r0 : r0 + 4, 1:129],
                        dx3,
                        xc[:, r0 : r0 + 4, 1:129],
                        Alu.add,
                    )
                    # fp16 shadow of the updated tile
                    nc.vector.tensor_copy(
                        xn6[:, r0 : r0 + 4, 1:129], xn[:, r0 : r0 + 4, 1:129]
                    )
                    if s + 1 < STEPS and j in TRIGGER:
                        srows = {0: (1, 5), 1: (5, 17), 2: (17, 25), 3: (25, 33)}[TRIGGER[j]]
                        emit_slice(s, TRIGGER[j], srows)

                prev = None
                for j in JORD:
                    r0 = 4 * j + 1
                    hts = [
                        hpool.tile([128, TW], f32, tag="h", name=f"h_{s}_{j}_{g}")
                        for g in range(G)
                    ]
                    # mm1: 3 dy-matmuls per group (K padded to 128 for full
                    # rate); dy shift via the rhs AP, dx via the stack rows
                    for di, dy in enumerate((-1, 0, 1)):
                        for g in range(G):
                            rhs = stks[g][0:128, r0 + dy : r0 + dy + 4, 0:128]
                            lhsT = A[0:128, 128 * (dy + 1) : 128 * (dy + 2)]
                            nc.tensor.matmul(
                                hts[g][:, :],
                                lhsT,
                                rhs,
                                start=(di == 0),
                                stop=(di == 2),
                                tile_position=(0, 0),
                            )
                    hss = []
                    for g in range(G):
                        ht = hts[g]
                        hs = hsb.tile(
                            [128, TW], f16, tag="hsb", name=f"hs_{s}_{j}_{g}"
                        )
                        # h' = relu(h + b1 - M); dy=0 fire row added fire*M
                        nc.scalar.activation(
                            hs[:, :288],
                            ht[:, :288],
                            Act.Relu,
                            bias=b1c[:],
                        )
                        nc.vector.scalar_tensor_tensor(
                            hs[:, 288:],
                            ht[:, 288:],
                            b1c[:],
                            zeros[:, :224],
                            Alu.add,
                            Alu.max,
                        )
                        hss.append(hs)
                    if prev is not None:
                        mm2_update(*prev)
                    prev = (j, hss)
                mm2_update(*prev)
                if s + 1 < STEPS:
                    emit_fire_edges(s)

            xfin = xf32[STEPS % 2][:].rearrange("p (r w) -> p r w", w=PITCH)
            xo3 = xout.rearrange("p (r w) -> p r w", w=128)
            nc.sync.dma_start(xo3[:, :, :], xfin[:, 1:33, 1:129])

    nc.compile()
    return nc


def _get_module():
    if "nc" not in _CACHE:
        _CACHE["nc"] = _build_module()
    return _CACHE["nc"]


def _prep_weights(w1, w2, W1, b1, W2):
    A = np.zeros((9, HID, C), np.float32)
    for t in range(9):
        dy, dxx = t // 3 - 1, t % 3 - 1
        A[t] = (
            W1[:, 24:48] * w1[dy + 1, dxx + 1, 0][None, :]
            + W1[:, 48:72] * w2[dy + 1, dxx + 1, 0][None, :]
        )
    A[4] += W1[:, :24]
    apack = np.zeros((128, 384), np.float32)
    for d in range(3):
        for dyi in range(3):
            t = dyi * 3 + d
            apack[24 * d : 24 * d + 24, 128 * dyi : 128 * dyi + 128] = A[t].T
    apack[72, 128:256] = M_FIRE
    w2pk = np.zeros((128, 32), np.float32)
    w2pk[:, NIC:C] = W2[NIC:C].T
    b1c = (b1 - M_FIRE).reshape(128, 1).astype(np.float32)
    return apack.astype(np.float16), w2pk.astype(np.float16), b1c


def _pack_x(ximg):
    """[128,128,24] image -> [128, FB] haloed channel-major."""
    xin = np.zeros((128, FB), np.float32)
    cols = (np.arange(-1, 129)) % WID
    for g in range(G):
        rows = (np.arange(-1, 33) + 32 * g) % H
        blk = ximg[rows][:, cols, :]  # [34, 130, 24]
        buf = np.zeros((24, GROWS, PITCH), np.float32)
        buf[:, :, :130] = np.transpose(blk, (2, 0, 1))
        xin[32 * g : 32 * g + 24] = buf.reshape(24, FB)
    return xin


def _unpack_x(xo):
    """[128, 4096] -> [128,128,24] image."""
    img = np.empty((H, WID, C), np.float32)
    for g in range(G):
        blk = xo[32 * g : 32 * g + 24].reshape(24, RG, WID)
        img[32 * g : 32 * g + 32] = np.transpose(blk, (1, 2, 0))
    return img


def _build_stack0(x16, uin):
    """Host: step-0 stacks, one [128, FB] block per group."""
    stkin = np.zeros((128, 4 * FB), np.float16)
    fire0 = (uin < 0.5).astype(np.float16)
    for g in range(G):
        blk = stkin[:, g * FB : (g + 1) * FB]
        for d in range(3):
            blk[24 * d : 24 * d + 24, : FB - d] = x16[
                32 * g : 32 * g + 24, d : FB
            ]
        fr = fire0[32 * g].reshape(32, 128)
        f2 = blk[72].reshape(GROWS, PITCH)
        f2[1:33, 0:128] = fr
    return stkin


def _make_in_maps(x, w1, w2, W1, b1, W2, rand_u):
    apack, w2pk, b1c = _prep_weights(w1, w2, W1, b1, W2)
    in_maps = []
    for b in range(B):
        u = rand_u[:, b, :, :, 0].reshape(STEPS, H * WID).astype(np.float32)
        uin = np.zeros((128, 4096), np.float32)
        for g in range(G):
            for s in range(STEPS):
                uin[32 * g + s] = u[s, g * 4096 : (g + 1) * 4096]
        xin = _pack_x(np.asarray(x[b], np.float32))
        x16 = xin.astype(np.float16)
        in_maps.append(
            {
                "xin": xin,
                "uin": uin,
                "apack": apack,
                "w2p": w2pk,
                "b1col": b1c,
                "x16in": x16,
                "stkin": _build_stack0(x16, uin),
            }
        )
    return in_maps


def kernel(x, w1, w2, W1, b1, W2, rand_u, steps, **kw):
    from concourse.bass_utils import run_bass_kernel_spmd

    assert int(steps) == STEPS
    x = np.asarray(x, np.float32)
    in_maps = _make_in_maps(
        x,
        np.asarray(w1, np.float32),
        np.asarray(w2, np.float32),
        np.asarray(W1, np.float32),
        np.asarray(b1, np.float32),
        np.asarray(W2, np.float32),
        np.asarray(rand_u, np.float32),
    )
    nc = _get_module()
    res = run_bass_kernel_spmd(nc, in_maps, core_ids=list(range(B)))
    _CACHE["last_results"] = res
    out = np.empty((B, H, WID, C), np.float32)
    for b in range(B):
        out[b] = _unpack_x(res.results[b]["xout"])
    return out

